# revision 42
# baseline (speedup 1.0000x reference)
"""Causal multi-head attention on 8 Trainium2 NeuronCores.

Sharding: data-parallel over batch (B=4) x tensor-parallel over heads
(16 heads -> 2 groups of 8). Core (b, hg) computes, for batch b and its
8 heads: qkv projection (column-parallel), causal attention, and the
row-parallel slice of the output projection. Host sums the two partial
projections per batch and adds b_proj.

Device layout avoids all on-chip transposes:
  - host supplies x^T (augmented with a ones row so b_attn folds into the
    matmul as a K=1152 contraction)
  - q,k are produced transposed [d, tok]; v natural [tok, d]
  - scores are computed transposed s^T[key, q]; causal mask applied as an
    additive -1e30 (slices of one precomputed sliding mask) on PSUM before
    a single wide exp on ACT
  - attn@v uses v augmented with a ones column, so the softmax denominator
    Z lands in psum row 64 of the same matmul
  - normalization multiplies by recip(Z) broadcast across partitions via a
    K=1 matmul with a ones vector
Matmul operands are float32r (PE full rate; ~1e-4 component error).
"""

from contextlib import ExitStack

import numpy as np

import concourse.bacc as bacc
import concourse.bass as bass
import concourse.mybir as mybir
import concourse.tile as tile
from concourse.bass_utils import run_bass_kernel_spmd

B, S, C = 4, 2048, 1024
H, D = 16, 64
HG = 8            # heads per core
CG = HG * D       # 512 channels per head-group
KAUG = 1152      # 1024 + 1 (bias row) padded to 9*128
NCH = KAUG // 128
P = 128
TT = 512          # token tile for stage 1 / q tile for stage 2
NQT = S // TT     # 4
NKC = S // P      # 16 key chunks
F32 = mybir.dt.float32
MMDT = mybir.dt.float32r  # matmul operand dtype (F32 for exact, float32r for speed)
ALU = mybir.AluOpType


def _build():
    nc = bacc.Bacc(None, target_bir_lowering=False, debug=False)
    xaT = nc.dram_tensor("xaT", [KAUG, S], MMDT, kind="ExternalInput")
    wq = nc.dram_tensor("wq", [KAUG, CG], MMDT, kind="ExternalInput")
    wk = nc.dram_tensor("wk", [KAUG, CG], MMDT, kind="ExternalInput")
    wv = nc.dram_tensor("wv", [KAUG, CG], MMDT, kind="ExternalInput")
    wp = nc.dram_tensor("wp", [CG, C], MMDT, kind="ExternalInput")
    msk = nc.dram_tensor("msk", [P, 2, 2 * TT], F32, kind="ExternalInput")
    onesd = nc.dram_tensor("onesd", [P, P], MMDT, kind="ExternalInput")
    y = nc.dram_tensor("y", [S, C], F32, kind="ExternalOutput")

    with tile.TileContext(nc) as tc, ExitStack() as ctx:
        consts = ctx.enter_context(tc.tile_pool(name="consts", bufs=1))
        qkpool = ctx.enter_context(tc.tile_pool(name="qkpool", bufs=1))

        mask_sb = consts.tile([P, 2, 2 * TT], F32)
        nc.sync.dma_start(mask_sb[:], msk[:])

        qT_sb = qkpool.tile([P, CG // P, S], MMDT)
        kT_sb = qkpool.tile([P, CG // P, S], MMDT)
        v_sb = qkpool.tile([P, NKC, HG, D + 1], MMDT)
        nc.sync.dma_start(
            v_sb[:, :, :, D : D + 1],
            onesd[:, :].rearrange("p (a b o) -> p a b o", a=NKC, b=HG),
        )

        # ---- stage 1: qkv projection ----
        with (
            tc.tile_pool(name="wqkv", bufs=1) as wpool,
            tc.tile_pool(name="xa", bufs=2) as xpool,
            tc.tile_pool(name="ps1", bufs=3, space="PSUM") as ps1,
        ):
            wq_sb = wpool.tile([P, NCH, CG], MMDT)
            wk_sb = wpool.tile([P, NCH, CG], MMDT)
            wv_sb = wpool.tile([P, NCH, CG], MMDT)
            # per-chunk DMAs, interleaved with the first x tile, so the first
            # matmuls start as soon as chunk 0 lands instead of after 9 MB
            xa0 = xpool.tile([P, NCH, TT], MMDT, tag="xa")
            for ko in range(NCH):
                nc.sync.dma_start(xa0[:, ko, :], xaT[ko * P : (ko + 1) * P, 0:TT])
                for w_sb, w_dr in ((wq_sb, wq), (wk_sb, wk), (wv_sb, wv)):
                    nc.sync.dma_start(
                        w_sb[:, ko, :], w_dr[ko * P : (ko + 1) * P, :]
                    )

            for tt in range(NQT):
                if tt == 0:
                    xa = xa0
                else:
                    xa = xpool.tile([P, NCH, TT], MMDT, tag="xa")
                    for ko in range(NCH):
                        nc.sync.dma_start(
                            xa[:, ko, :],
                            xaT[ko * P : (ko + 1) * P, tt * TT : (tt + 1) * TT],
                        )
                for w_sb, out_sb in ((wq_sb, qT_sb), (wk_sb, kT_sb)):
                    for cb in range(CG // P):
                        acc = ps1.tile([P, TT], F32, tag="mm")
                        for ko in range(NCH):
                            nc.tensor.matmul(
                                acc[:],
                                w_sb[:, ko, cb * P : (cb + 1) * P],
                                xa[:, ko, :],
                                start=(ko == 0),
                                stop=(ko == NCH - 1),
                            )
                        nc.vector.tensor_copy(
                            out_sb[:, cb, tt * TT : (tt + 1) * TT], acc[:]
                        )
                for tb in range(TT // P):
                    acc = ps1.tile([P, CG], F32, tag="mm")
                    for ko in range(NCH):
                        nc.tensor.matmul(
                            acc[:],
                            xa[:, ko, tb * P : (tb + 1) * P],
                            wv_sb[:, ko, :],
                            start=(ko == 0),
                            stop=(ko == NCH - 1),
                        )
                    nc.vector.tensor_copy(
                        v_sb[:, tt * 4 + tb, :, 0:D],
                        acc[:].rearrange("p (h d) -> p h d", h=HG),
                    )

        # ---- phase B pools (reuse the stage-1 SBUF/PSUM space) ----
        wp_pool = ctx.enter_context(tc.tile_pool(name="wp_pool", bufs=1))
        spool = ctx.enter_context(tc.tile_pool(name="spool", bufs=2))
        opool = ctx.enter_context(tc.tile_pool(name="opool", bufs=2))
        zpool = ctx.enter_context(tc.tile_pool(name="zpool", bufs=2))
        ypool = ctx.enter_context(tc.tile_pool(name="ypool", bufs=3))
        ps2 = ctx.enter_context(tc.tile_pool(name="ps2", bufs=1, space="PSUM"))
        psy = ctx.enter_context(tc.tile_pool(name="psy", bufs=2, space="PSUM"))
        pso = ctx.enter_context(tc.tile_pool(name="pso", bufs=2, space="PSUM"))

        wp_sb = wp_pool.tile([64, HG, C], MMDT)
        nc.sync.dma_start(wp_sb[:], wp.rearrange("(h p) n -> p h n", p=64))

        # ---- stage 2+3: attention + output projection, per q tile ----
        # heads are processed in pairs (hA at partitions 0-63, hB at 64-127):
        # their score matmuls are adjacent K=64 matmuls in different PE
        # row-groups, so the array runs them concurrently; pairing also gives
        # the scheduler two independent chains to hide exp/mask latency.
        for qt in range(NQT):
            oT = opool.tile([64, HG, TT], MMDT)
            nvalid = 4 * qt + 4
            for hp in range(HG // 2):
                hA, hB = 2 * hp, 2 * hp + 1
                qsA = qT_sb[0:64, hp, qt * TT : (qt + 1) * TT]
                qsB = qT_sb[64:128, hp, qt * TT : (qt + 1) * TT]
                acc_oA = pso.tile([D + 1, TT], F32, tag="o")
                acc_oB = pso.tile([D + 1, TT], F32, tag="o")
                for kc2 in range(nvalid // 2):
                    scA = ps2.tile([P, 2 * TT], F32, tag="scA")
                    scB = ps2.tile([P, 2 * TT], F32, tag="scB")
                    exA = spool.tile([P, 2 * TT], MMDT, tag="exA")
                    exB = spool.tile([P, 2 * TT], MMDT, tag="exB")
                    for j in range(2):
                        kc = 2 * kc2 + j
                        kslc = slice(kc * P, (kc + 1) * P)
                        nc.tensor.matmul(
                            scA[:, j * TT : (j + 1) * TT],
                            kT_sb[0:64, hp, kslc], qsA,
                            start=True, stop=True,
                        )
                        nc.tensor.matmul(
                            scB[:, j * TT : (j + 1) * TT],
                            kT_sb[64:128, hp, kslc], qsB,
                            start=True, stop=True,
                        )
                    if 2 * kc2 >= 4 * qt:  # diagonal pair: additive causal mask
                        dp = (2 * kc2 - 4 * qt) // 2  # 0 or 1
                        nc.vector.tensor_add(scA[:], scA[:], mask_sb[:, dp, :])
                        nc.vector.tensor_add(scB[:], scB[:], mask_sb[:, dp, :])
                    nc.scalar.activation(
                        exA[:], scA[:], mybir.ActivationFunctionType.Exp,
                        scale=1.0 / np.sqrt(D),
                    )
                    nc.scalar.activation(
                        exB[:], scB[:], mybir.ActivationFunctionType.Exp,
                        scale=1.0 / np.sqrt(D),
                    )
                    for j in range(2):
                        kc = 2 * kc2 + j
                        nc.tensor.matmul(
                            acc_oA[:], v_sb[:, kc, hA, :],
                            exA[:, j * TT : (j + 1) * TT],
                            start=(kc == 0), stop=(kc == nvalid - 1),
                        )
                        nc.tensor.matmul(
                            acc_oB[:], v_sb[:, kc, hB, :],
                            exB[:, j * TT : (j + 1) * TT],
                            start=(kc == 0), stop=(kc == nvalid - 1),
                        )
                for h, acc_o in ((hA, acc_oA), (hB, acc_oB)):
                    rz = zpool.tile([1, TT], F32, tag="rz")
                    nc.vector.reciprocal(rz[:], acc_o[D : D + 1, :])
                    bc = zpool.tile([64, TT], F32, tag="bc")
                    nc.gpsimd.partition_broadcast(bc[:], rz[:])
                    nc.vector.tensor_mul(oT[:, h, :], acc_o[0:D, :], bc[:])

            for qb in range(TT // P):
                for nt in range(C // TT):
                    acc_y = psy.tile([P, TT], F32)
                    for h in range(HG):
                        nc.tensor.matmul(
                            acc_y[:],
                            oT[:, h, qb * P : (qb + 1) * P],
                            wp_sb[:, h, nt * TT : (nt + 1) * TT],
                            start=(h == 0),
                            stop=(h == HG - 1),
                        )
                    ysb = ypool.tile([P, TT], F32)
                    nc.vector.tensor_copy(ysb[:], acc_y[:])
                    nc.sync.dma_start(
                        y[
                            qt * TT + qb * P : qt * TT + (qb + 1) * P,
                            nt * TT : (nt + 1) * TT,
                        ],
                        ysb[:],
                    )
    nc.compile()
    return nc


_NC_CACHE = {}


def _get_nc():
    key = str(MMDT)
    if key not in _NC_CACHE:
        _NC_CACHE[key] = _build()
    return _NC_CACHE[key]


def _make_mask():
    # additive causal pair-masks: msk[:, dp, :] covers a [128, 1024] score
    # tile holding key-chunk pair (delta, delta+128) with delta = 512*dp
    # relative to the q-tile start. 0 where key <= query, -1e30 otherwise.
    r = np.arange(P)[:, None]
    c = np.arange(TT)[None, :]
    out = np.empty((P, 2, 2 * TT), dtype=np.float32)
    # pair dp covers key chunks with delta = 2*dp*128 and (2*dp+1)*128
    for dp in range(2):
        dj0 = (2 * dp) * P
        dj1 = (2 * dp + 1) * P
        out[:, dp, :TT] = np.where(c >= r + dj0, 0.0, -1e30)
        out[:, dp, TT:] = np.where(c >= r + dj1, 0.0, -1e30)
    return out


def _prep_inputs(x, W_attn, b_attn, W_proj):
    mask = _make_mask()
    in_maps = []
    for b in range(B):
        xaT = np.zeros((KAUG, S), dtype=np.float32)
        xaT[:C] = np.ascontiguousarray(x[b].T)
        xaT[C] = 1.0
        for hg in range(2):
            cs = hg * CG
            m = {"xaT": xaT, "msk": mask,
                 "onesd": np.ones((P, P), dtype=np.float32)}
            for name, off in (("wq", 0), ("wk", C), ("wv", 2 * C)):
                w = np.zeros((KAUG, CG), dtype=np.float32)
                w[:C] = W_attn[:, off + cs : off + cs + CG]
                w[C] = b_attn[off + cs : off + cs + CG]
                m[name] = w
            m["wp"] = np.ascontiguousarray(W_proj[cs : cs + CG, :])
            in_maps.append(m)
    return in_maps


def _run(x, W_attn, b_attn, W_proj, b_proj, trace=False, **run_kwargs):
    x = np.asarray(x, dtype=np.float32)
    W_attn = np.asarray(W_attn, dtype=np.float32)
    b_attn = np.asarray(b_attn, dtype=np.float32)
    W_proj = np.asarray(W_proj, dtype=np.float32)
    b_proj = np.asarray(b_proj, dtype=np.float32)

    nc = _get_nc()
    in_maps = _prep_inputs(x, W_attn, b_attn, W_proj)
    res = run_bass_kernel_spmd(
        nc, in_maps, core_ids=list(range(B * 2)), trace=trace, **run_kwargs
    )
    out = np.empty((B, S, C), dtype=np.float32)
    for b in range(B):
        out[b] = res.results[2 * b]["y"] + res.results[2 * b + 1]["y"] + b_proj
    return out, res


def kernel(x, W_attn, b_attn, W_proj, b_proj):
    out, _ = _run(x, W_attn, b_attn, W_proj, b_proj)
    return out


# revision 47
# speedup vs baseline: 1.0220x; 1.0220x over previous
"""Causal multi-head attention on 8 Trainium2 NeuronCores.

Sharding: data-parallel over batch (B=4) x tensor-parallel over heads
(16 heads -> 2 groups of 8). Core (b, hg) computes, for batch b and its
8 heads: qkv projection (column-parallel), causal attention, and the
row-parallel slice of the output projection. Host sums the two partial
projections per batch and adds b_proj.

Device layout avoids all on-chip transposes:
  - host supplies x^T (augmented with a ones row so b_attn folds into the
    matmul as a K=1152 contraction)
  - q,k are produced transposed [d, tok]; v natural [tok, d]
  - scores are computed transposed s^T[key, q]; causal mask applied as an
    additive -1e30 (slices of one precomputed sliding mask) on PSUM before
    a single wide exp on ACT
  - attn@v uses v augmented with a ones column, so the softmax denominator
    Z lands in psum row 64 of the same matmul
  - normalization multiplies by recip(Z) broadcast across partitions via a
    K=1 matmul with a ones vector
Matmul operands are float32r (PE full rate; ~1e-4 component error).
"""

from contextlib import ExitStack

import numpy as np

import concourse.bacc as bacc
import concourse.bass as bass
import concourse.mybir as mybir
import concourse.tile as tile
from concourse.bass_utils import run_bass_kernel_spmd

B, S, C = 4, 2048, 1024
H, D = 16, 64
HG = 8            # heads per core
CG = HG * D       # 512 channels per head-group
KAUG = 1152      # 1024 + 1 (bias row) padded to 9*128
NCH = KAUG // 128
P = 128
TT = 512          # token tile for stage 1 / q tile for stage 2
NQT = S // TT     # 4
NKC = S // P      # 16 key chunks
F32 = mybir.dt.float32
MMDT = mybir.dt.float32r  # matmul operand dtype (F32 for exact, float32r for speed)
ALU = mybir.AluOpType


def _build():
    nc = bacc.Bacc(None, target_bir_lowering=False, debug=False)
    xaT = nc.dram_tensor("xaT", [KAUG, S], MMDT, kind="ExternalInput")
    wq = nc.dram_tensor("wq", [KAUG, CG], MMDT, kind="ExternalInput")
    wk = nc.dram_tensor("wk", [KAUG, CG], MMDT, kind="ExternalInput")
    wv = nc.dram_tensor("wv", [KAUG, CG], MMDT, kind="ExternalInput")
    wp = nc.dram_tensor("wp", [CG, C], MMDT, kind="ExternalInput")
    msk = nc.dram_tensor("msk", [P, 2, 2 * TT], F32, kind="ExternalInput")
    onesd = nc.dram_tensor("onesd", [P, P], MMDT, kind="ExternalInput")
    y = nc.dram_tensor("y", [S, C], F32, kind="ExternalOutput")

    with tile.TileContext(nc) as tc, ExitStack() as ctx:
        consts = ctx.enter_context(tc.tile_pool(name="consts", bufs=1))
        qkpool = ctx.enter_context(tc.tile_pool(name="qkpool", bufs=1))

        mask_sb = consts.tile([P, 2, 2 * TT], F32)
        nc.sync.dma_start(mask_sb[:], msk[:])

        qT_sb = qkpool.tile([P, CG // P, S], MMDT)
        kT_sb = qkpool.tile([P, CG // P, S], MMDT)
        v_sb = qkpool.tile([P, NKC, HG, D + 1], MMDT)
        nc.sync.dma_start(
            v_sb[:, :, :, D : D + 1],
            onesd[:, :].rearrange("p (a b o) -> p a b o", a=NKC, b=HG),
        )

        # ---- stage 1: qkv projection ----
        with (
            tc.tile_pool(name="wqkv", bufs=1) as wpool,
            tc.tile_pool(name="xa", bufs=2) as xpool,
            tc.tile_pool(name="ps1", bufs=8, space="PSUM") as ps1,
        ):
            wq_sb = wpool.tile([P, NCH, CG], MMDT)
            wk_sb = wpool.tile([P, NCH, CG], MMDT)
            wv_sb = wpool.tile([P, NCH, CG], MMDT)
            # per-chunk DMAs, interleaved with the first x tile, so the first
            # matmuls start as soon as chunk 0 lands instead of after 9 MB
            xa0 = xpool.tile([P, NCH, TT], MMDT, tag="xa")
            for ko in range(NCH):
                nc.sync.dma_start(xa0[:, ko, :], xaT[ko * P : (ko + 1) * P, 0:TT])
                for w_sb, w_dr in ((wq_sb, wq), (wk_sb, wk), (wv_sb, wv)):
                    nc.sync.dma_start(
                        w_sb[:, ko, :], w_dr[ko * P : (ko + 1) * P, :]
                    )

            for tt in range(NQT):
                if tt == 0:
                    xa = xa0
                else:
                    xa = xpool.tile([P, NCH, TT], MMDT, tag="xa")
                    for ko in range(NCH):
                        nc.sync.dma_start(
                            xa[:, ko, :],
                            xaT[ko * P : (ko + 1) * P, tt * TT : (tt + 1) * TT],
                        )
                for w_sb, out_sb in ((wq_sb, qT_sb), (wk_sb, kT_sb)):
                    for cb in range(CG // P):
                        acc = ps1.tile([P, TT], F32, tag="mm")
                        for ko in range(NCH):
                            nc.tensor.matmul(
                                acc[:],
                                w_sb[:, ko, cb * P : (cb + 1) * P],
                                xa[:, ko, :],
                                start=(ko == 0),
                                stop=(ko == NCH - 1),
                            )
                        nc.vector.tensor_copy(
                            out_sb[:, cb, tt * TT : (tt + 1) * TT], acc[:]
                        )
                for tb in range(TT // P):
                    acc = ps1.tile([P, CG], F32, tag="mm")
                    for ko in range(NCH):
                        nc.tensor.matmul(
                            acc[:],
                            xa[:, ko, tb * P : (tb + 1) * P],
                            wv_sb[:, ko, :],
                            start=(ko == 0),
                            stop=(ko == NCH - 1),
                        )
                    nc.vector.tensor_copy(
                        v_sb[:, tt * 4 + tb, :, 0:D],
                        acc[:].rearrange("p (h d) -> p h d", h=HG),
                    )

        # ---- phase B pools (reuse the stage-1 SBUF/PSUM space) ----
        wp_pool = ctx.enter_context(tc.tile_pool(name="wp_pool", bufs=1))
        spool = ctx.enter_context(tc.tile_pool(name="spool", bufs=2))
        opool = ctx.enter_context(tc.tile_pool(name="opool", bufs=2))
        zpool = ctx.enter_context(tc.tile_pool(name="zpool", bufs=2))
        ypool = ctx.enter_context(tc.tile_pool(name="ypool", bufs=3))
        ps2 = ctx.enter_context(tc.tile_pool(name="ps2", bufs=1, space="PSUM"))
        psy = ctx.enter_context(tc.tile_pool(name="psy", bufs=2, space="PSUM"))
        pso = ctx.enter_context(tc.tile_pool(name="pso", bufs=2, space="PSUM"))

        wp_sb = wp_pool.tile([64, HG, C], MMDT)
        nc.sync.dma_start(wp_sb[:], wp.rearrange("(h p) n -> p h n", p=64))

        # ---- stage 2+3: attention + output projection, per q tile ----
        # heads are processed in pairs (hA at partitions 0-63, hB at 64-127):
        # their score matmuls are adjacent K=64 matmuls in different PE
        # row-groups, so the array runs them concurrently; pairing also gives
        # the scheduler two independent chains to hide exp/mask latency.
        for qt in range(NQT):
            oT = opool.tile([64, HG, TT], MMDT)
            nvalid = 4 * qt + 4
            for hp in range(HG // 2):
                hA, hB = 2 * hp, 2 * hp + 1
                qsA = qT_sb[0:64, hp, qt * TT : (qt + 1) * TT]
                qsB = qT_sb[64:128, hp, qt * TT : (qt + 1) * TT]
                acc_oA = pso.tile([D + 1, TT], F32, tag="o")
                acc_oB = pso.tile([D + 1, TT], F32, tag="o")
                for kc2 in range(nvalid // 2):
                    scA = ps2.tile([P, 2 * TT], F32, tag="scA")
                    scB = ps2.tile([P, 2 * TT], F32, tag="scB")
                    exA = spool.tile([P, 2 * TT], MMDT, tag="exA")
                    exB = spool.tile([P, 2 * TT], MMDT, tag="exB")
                    for j in range(2):
                        kc = 2 * kc2 + j
                        kslc = slice(kc * P, (kc + 1) * P)
                        nc.tensor.matmul(
                            scA[:, j * TT : (j + 1) * TT],
                            kT_sb[0:64, hp, kslc], qsA,
                            start=True, stop=True,
                        )
                        nc.tensor.matmul(
                            scB[:, j * TT : (j + 1) * TT],
                            kT_sb[64:128, hp, kslc], qsB,
                            start=True, stop=True,
                        )
                    if 2 * kc2 >= 4 * qt:  # diagonal pair: additive causal mask
                        dp = (2 * kc2 - 4 * qt) // 2  # 0 or 1
                        nc.vector.tensor_add(scA[:], scA[:], mask_sb[:, dp, :])
                        nc.vector.tensor_add(scB[:], scB[:], mask_sb[:, dp, :])
                    nc.scalar.activation(
                        exA[:], scA[:], mybir.ActivationFunctionType.Exp,
                        scale=1.0 / np.sqrt(D),
                    )
                    nc.scalar.activation(
                        exB[:], scB[:], mybir.ActivationFunctionType.Exp,
                        scale=1.0 / np.sqrt(D),
                    )
                    for j in range(2):
                        kc = 2 * kc2 + j
                        nc.tensor.matmul(
                            acc_oA[:], v_sb[:, kc, hA, :],
                            exA[:, j * TT : (j + 1) * TT],
                            start=(kc == 0), stop=(kc == nvalid - 1),
                        )
                        nc.tensor.matmul(
                            acc_oB[:], v_sb[:, kc, hB, :],
                            exB[:, j * TT : (j + 1) * TT],
                            start=(kc == 0), stop=(kc == nvalid - 1),
                        )
                for h, acc_o in ((hA, acc_oA), (hB, acc_oB)):
                    rz = zpool.tile([1, TT], F32, tag="rz")
                    nc.vector.reciprocal(rz[:], acc_o[D : D + 1, :])
                    bc = zpool.tile([64, TT], F32, tag="bc")
                    nc.gpsimd.partition_broadcast(bc[:], rz[:])
                    nc.vector.tensor_mul(oT[:, h, :], acc_o[0:D, :], bc[:])

            for qb in range(TT // P):
                for nt in range(C // TT):
                    acc_y = psy.tile([P, TT], F32)
                    for h in range(HG):
                        nc.tensor.matmul(
                            acc_y[:],
                            oT[:, h, qb * P : (qb + 1) * P],
                            wp_sb[:, h, nt * TT : (nt + 1) * TT],
                            start=(h == 0),
                            stop=(h == HG - 1),
                        )
                    ysb = ypool.tile([P, TT], F32)
                    nc.vector.tensor_copy(ysb[:], acc_y[:])
                    nc.sync.dma_start(
                        y[
                            qt * TT + qb * P : qt * TT + (qb + 1) * P,
                            nt * TT : (nt + 1) * TT,
                        ],
                        ysb[:],
                    )
    nc.compile()
    return nc


_NC_CACHE = {}


def _get_nc():
    key = str(MMDT)
    if key not in _NC_CACHE:
        _NC_CACHE[key] = _build()
    return _NC_CACHE[key]


def _make_mask():
    # additive causal pair-masks: msk[:, dp, :] covers a [128, 1024] score
    # tile holding key-chunk pair (delta, delta+128) with delta = 512*dp
    # relative to the q-tile start. 0 where key <= query, -1e30 otherwise.
    r = np.arange(P)[:, None]
    c = np.arange(TT)[None, :]
    out = np.empty((P, 2, 2 * TT), dtype=np.float32)
    # pair dp covers key chunks with delta = 2*dp*128 and (2*dp+1)*128
    for dp in range(2):
        dj0 = (2 * dp) * P
        dj1 = (2 * dp + 1) * P
        out[:, dp, :TT] = np.where(c >= r + dj0, 0.0, -1e30)
        out[:, dp, TT:] = np.where(c >= r + dj1, 0.0, -1e30)
    return out


def _prep_inputs(x, W_attn, b_attn, W_proj):
    mask = _make_mask()
    in_maps = []
    for b in range(B):
        xaT = np.zeros((KAUG, S), dtype=np.float32)
        xaT[:C] = np.ascontiguousarray(x[b].T)
        xaT[C] = 1.0
        for hg in range(2):
            cs = hg * CG
            m = {"xaT": xaT, "msk": mask,
                 "onesd": np.ones((P, P), dtype=np.float32)}
            for name, off in (("wq", 0), ("wk", C), ("wv", 2 * C)):
                w = np.zeros((KAUG, CG), dtype=np.float32)
                w[:C] = W_attn[:, off + cs : off + cs + CG]
                w[C] = b_attn[off + cs : off + cs + CG]
                m[name] = w
            m["wp"] = np.ascontiguousarray(W_proj[cs : cs + CG, :])
            in_maps.append(m)
    return in_maps


def _run(x, W_attn, b_attn, W_proj, b_proj, trace=False, **run_kwargs):
    x = np.asarray(x, dtype=np.float32)
    W_attn = np.asarray(W_attn, dtype=np.float32)
    b_attn = np.asarray(b_attn, dtype=np.float32)
    W_proj = np.asarray(W_proj, dtype=np.float32)
    b_proj = np.asarray(b_proj, dtype=np.float32)

    nc = _get_nc()
    in_maps = _prep_inputs(x, W_attn, b_attn, W_proj)
    res = run_bass_kernel_spmd(
        nc, in_maps, core_ids=list(range(B * 2)), trace=trace, **run_kwargs
    )
    out = np.empty((B, S, C), dtype=np.float32)
    for b in range(B):
        out[b] = res.results[2 * b]["y"] + res.results[2 * b + 1]["y"] + b_proj
    return out, res


def kernel(x, W_attn, b_attn, W_proj, b_proj):
    out, _ = _run(x, W_attn, b_attn, W_proj, b_proj)
    return out


# revision 48
# speedup vs baseline: 1.0424x; 1.0200x over previous
"""Causal multi-head attention on 8 Trainium2 NeuronCores.

Sharding: data-parallel over batch (B=4) x tensor-parallel over heads
(16 heads -> 2 groups of 8). Core (b, hg) computes, for batch b and its
8 heads: qkv projection (column-parallel), causal attention, and the
row-parallel slice of the output projection. Host sums the two partial
projections per batch and adds b_proj.

Device layout avoids all on-chip transposes:
  - host supplies x^T (augmented with a ones row so b_attn folds into the
    matmul as a K=1152 contraction)
  - q,k are produced transposed [d, tok]; v natural [tok, d]
  - scores are computed transposed s^T[key, q]; causal mask applied as an
    additive -1e30 (slices of one precomputed sliding mask) on PSUM before
    a single wide exp on ACT
  - attn@v uses v augmented with a ones column, so the softmax denominator
    Z lands in psum row 64 of the same matmul
  - normalization multiplies by recip(Z) broadcast across partitions via a
    K=1 matmul with a ones vector
Matmul operands are float32r (PE full rate; ~1e-4 component error).
"""

from contextlib import ExitStack

import numpy as np

import concourse.bacc as bacc
import concourse.bass as bass
import concourse.mybir as mybir
import concourse.tile as tile
from concourse.bass_utils import run_bass_kernel_spmd

B, S, C = 4, 2048, 1024
H, D = 16, 64
HG = 8            # heads per core
CG = HG * D       # 512 channels per head-group
KAUG = 1152      # 1024 + 1 (bias row) padded to 9*128
NCH = KAUG // 128
P = 128
TT = 512          # token tile for stage 1 / q tile for stage 2
NQT = S // TT     # 4
NKC = S // P      # 16 key chunks
F32 = mybir.dt.float32
MMDT = mybir.dt.float32r  # matmul operand dtype (F32 for exact, float32r for speed)
ALU = mybir.AluOpType


def _build():
    nc = bacc.Bacc(None, target_bir_lowering=False, debug=False)
    xaT = nc.dram_tensor("xaT", [KAUG, S], MMDT, kind="ExternalInput")
    wq = nc.dram_tensor("wq", [KAUG, CG], MMDT, kind="ExternalInput")
    wk = nc.dram_tensor("wk", [KAUG, CG], MMDT, kind="ExternalInput")
    wv = nc.dram_tensor("wv", [KAUG, CG], MMDT, kind="ExternalInput")
    wp = nc.dram_tensor("wp", [CG, C], MMDT, kind="ExternalInput")
    msk = nc.dram_tensor("msk", [P, 2, 2 * TT], F32, kind="ExternalInput")
    bqk = nc.dram_tensor("bqk", [P, 8], F32, kind="ExternalInput")
    onesd = nc.dram_tensor("onesd", [P, P], MMDT, kind="ExternalInput")
    y = nc.dram_tensor("y", [S, C], F32, kind="ExternalOutput")

    with tile.TileContext(nc) as tc, ExitStack() as ctx:
        consts = ctx.enter_context(tc.tile_pool(name="consts", bufs=1))
        qkpool = ctx.enter_context(tc.tile_pool(name="qkpool", bufs=1))

        mask_sb = consts.tile([P, 2, 2 * TT], F32)
        nc.sync.dma_start(mask_sb[:], msk[:])
        bqk_sb = consts.tile([P, 8], F32)
        nc.sync.dma_start(bqk_sb[:], bqk[:])

        qT_sb = qkpool.tile([P, CG // P, S], MMDT)
        kT_sb = qkpool.tile([P, CG // P, S], MMDT)
        v_sb = qkpool.tile([P, NKC, HG, D + 1], MMDT)
        nc.sync.dma_start(
            v_sb[:, :, :, D : D + 1],
            onesd[:, :].rearrange("p (a b o) -> p a b o", a=NKC, b=HG),
        )

        # ---- stage 1: qkv projection ----
        with (
            tc.tile_pool(name="wqkv", bufs=1) as wpool,
            tc.tile_pool(name="xa", bufs=2) as xpool,
            tc.tile_pool(name="ps1", bufs=8, space="PSUM") as ps1,
        ):
            NDC = 8  # data chunks; bias handled separately (pad rows skipped)
            wq_sb = wpool.tile([P, NDC, CG], MMDT)
            wk_sb = wpool.tile([P, NDC, CG], MMDT)
            wv_sb = wpool.tile([P, NDC, CG], MMDT)
            wvb = wpool.tile([1, CG], MMDT)
            nc.sync.dma_start(wvb[:], wv[C : C + 1, :])
            # per-chunk DMAs, interleaved with the first x tile, so the first
            # matmuls start as soon as chunk 0 lands instead of after 9 MB
            xa0 = xpool.tile([P, NDC, TT], MMDT, tag="xa")
            xb0 = xpool.tile([1, TT], MMDT, tag="xb")
            nc.sync.dma_start(xb0[:], xaT[C : C + 1, 0:TT])
            for ko in range(NDC):
                nc.sync.dma_start(xa0[:, ko, :], xaT[ko * P : (ko + 1) * P, 0:TT])
                for w_sb, w_dr in ((wq_sb, wq), (wk_sb, wk), (wv_sb, wv)):
                    nc.sync.dma_start(
                        w_sb[:, ko, :], w_dr[ko * P : (ko + 1) * P, :]
                    )

            for tt in range(NQT):
                if tt == 0:
                    xa, xb = xa0, xb0
                else:
                    xa = xpool.tile([P, NDC, TT], MMDT, tag="xa")
                    xb = xpool.tile([1, TT], MMDT, tag="xb")
                    nc.sync.dma_start(
                        xb[:], xaT[C : C + 1, tt * TT : (tt + 1) * TT]
                    )
                    for ko in range(NDC):
                        nc.sync.dma_start(
                            xa[:, ko, :],
                            xaT[ko * P : (ko + 1) * P, tt * TT : (tt + 1) * TT],
                        )
                for wi, (w_sb, out_sb) in enumerate(((wq_sb, qT_sb), (wk_sb, kT_sb))):
                    for cb in range(CG // P):
                        acc = ps1.tile([P, TT], F32, tag="mm")
                        for ko in range(NDC):
                            nc.tensor.matmul(
                                acc[:],
                                w_sb[:, ko, cb * P : (cb + 1) * P],
                                xa[:, ko, :],
                                start=(ko == 0),
                                stop=(ko == NDC - 1),
                            )
                        # bias varies along the partition (channel) axis here,
                        # so it folds into the eviction as a [P,1] scalar add
                        nc.vector.tensor_scalar_add(
                            out_sb[:, cb, tt * TT : (tt + 1) * TT], acc[:],
                            bqk_sb[:, 4 * wi + cb : 4 * wi + cb + 1],
                        )
                for tb in range(TT // P):
                    acc = ps1.tile([P, CG], F32, tag="mm")
                    for ko in range(NDC):
                        nc.tensor.matmul(
                            acc[:],
                            xa[:, ko, tb * P : (tb + 1) * P],
                            wv_sb[:, ko, :],
                            start=(ko == 0),
                            stop=False,
                        )
                    # v bias varies along the free axis: one K=1 matmul with
                    # the ones row of x^T adds it inside the accumulation
                    nc.tensor.matmul(
                        acc[:],
                        xb[0:1, tb * P : (tb + 1) * P],
                        wvb[:],
                        start=False,
                        stop=True,
                    )
                    nc.vector.tensor_copy(
                        v_sb[:, tt * 4 + tb, :, 0:D],
                        acc[:].rearrange("p (h d) -> p h d", h=HG),
                    )

        # ---- phase B pools (reuse the stage-1 SBUF/PSUM space) ----
        wp_pool = ctx.enter_context(tc.tile_pool(name="wp_pool", bufs=1))
        spool = ctx.enter_context(tc.tile_pool(name="spool", bufs=2))
        opool = ctx.enter_context(tc.tile_pool(name="opool", bufs=2))
        zpool = ctx.enter_context(tc.tile_pool(name="zpool", bufs=2))
        ypool = ctx.enter_context(tc.tile_pool(name="ypool", bufs=3))
        ps2 = ctx.enter_context(tc.tile_pool(name="ps2", bufs=1, space="PSUM"))
        psy = ctx.enter_context(tc.tile_pool(name="psy", bufs=2, space="PSUM"))
        pso = ctx.enter_context(tc.tile_pool(name="pso", bufs=2, space="PSUM"))

        wp_sb = wp_pool.tile([64, HG, C], MMDT)
        nc.sync.dma_start(wp_sb[:], wp.rearrange("(h p) n -> p h n", p=64))

        # ---- stage 2+3: attention + output projection, per q tile ----
        # heads are processed in pairs (hA at partitions 0-63, hB at 64-127):
        # their score matmuls are adjacent K=64 matmuls in different PE
        # row-groups, so the array runs them concurrently; pairing also gives
        # the scheduler two independent chains to hide exp/mask latency.
        for qt in range(NQT):
            oT = opool.tile([64, HG, TT], MMDT)
            nvalid = 4 * qt + 4
            for hp in range(HG // 2):
                hA, hB = 2 * hp, 2 * hp + 1
                qsA = qT_sb[0:64, hp, qt * TT : (qt + 1) * TT]
                qsB = qT_sb[64:128, hp, qt * TT : (qt + 1) * TT]
                acc_oA = pso.tile([D + 1, TT], F32, tag="o")
                acc_oB = pso.tile([D + 1, TT], F32, tag="o")
                for kc2 in range(nvalid // 2):
                    scA = ps2.tile([P, 2 * TT], F32, tag="scA")
                    scB = ps2.tile([P, 2 * TT], F32, tag="scB")
                    exA = spool.tile([P, 2 * TT], MMDT, tag="exA")
                    exB = spool.tile([P, 2 * TT], MMDT, tag="exB")
                    for j in range(2):
                        kc = 2 * kc2 + j
                        kslc = slice(kc * P, (kc + 1) * P)
                        nc.tensor.matmul(
                            scA[:, j * TT : (j + 1) * TT],
                            kT_sb[0:64, hp, kslc], qsA,
                            start=True, stop=True,
                        )
                        nc.tensor.matmul(
                            scB[:, j * TT : (j + 1) * TT],
                            kT_sb[64:128, hp, kslc], qsB,
                            start=True, stop=True,
                        )
                    if 2 * kc2 >= 4 * qt:  # diagonal pair: additive causal mask
                        dp = (2 * kc2 - 4 * qt) // 2  # 0 or 1
                        nc.vector.tensor_add(scA[:], scA[:], mask_sb[:, dp, :])
                        nc.vector.tensor_add(scB[:], scB[:], mask_sb[:, dp, :])
                    nc.scalar.activation(
                        exA[:], scA[:], mybir.ActivationFunctionType.Exp,
                        scale=1.0 / np.sqrt(D),
                    )
                    nc.scalar.activation(
                        exB[:], scB[:], mybir.ActivationFunctionType.Exp,
                        scale=1.0 / np.sqrt(D),
                    )
                    for j in range(2):
                        kc = 2 * kc2 + j
                        nc.tensor.matmul(
                            acc_oA[:], v_sb[:, kc, hA, :],
                            exA[:, j * TT : (j + 1) * TT],
                            start=(kc == 0), stop=(kc == nvalid - 1),
                        )
                        nc.tensor.matmul(
                            acc_oB[:], v_sb[:, kc, hB, :],
                            exB[:, j * TT : (j + 1) * TT],
                            start=(kc == 0), stop=(kc == nvalid - 1),
                        )
                for h, acc_o in ((hA, acc_oA), (hB, acc_oB)):
                    rz = zpool.tile([1, TT], F32, tag="rz")
                    nc.vector.reciprocal(rz[:], acc_o[D : D + 1, :])
                    bc = zpool.tile([64, TT], F32, tag="bc")
                    nc.gpsimd.partition_broadcast(bc[:], rz[:])
                    nc.vector.tensor_mul(oT[:, h, :], acc_o[0:D, :], bc[:])

            for qb in range(TT // P):
                for nt in range(C // TT):
                    acc_y = psy.tile([P, TT], F32)
                    for h in range(HG):
                        nc.tensor.matmul(
                            acc_y[:],
                            oT[:, h, qb * P : (qb + 1) * P],
                            wp_sb[:, h, nt * TT : (nt + 1) * TT],
                            start=(h == 0),
                            stop=(h == HG - 1),
                        )
                    ysb = ypool.tile([P, TT], F32)
                    nc.vector.tensor_copy(ysb[:], acc_y[:])
                    nc.sync.dma_start(
                        y[
                            qt * TT + qb * P : qt * TT + (qb + 1) * P,
                            nt * TT : (nt + 1) * TT,
                        ],
                        ysb[:],
                    )
    nc.compile()
    return nc


_NC_CACHE = {}


def _get_nc():
    key = str(MMDT)
    if key not in _NC_CACHE:
        _NC_CACHE[key] = _build()
    return _NC_CACHE[key]


def _make_mask():
    # additive causal pair-masks: msk[:, dp, :] covers a [128, 1024] score
    # tile holding key-chunk pair (delta, delta+128) with delta = 512*dp
    # relative to the q-tile start. 0 where key <= query, -1e30 otherwise.
    r = np.arange(P)[:, None]
    c = np.arange(TT)[None, :]
    out = np.empty((P, 2, 2 * TT), dtype=np.float32)
    # pair dp covers key chunks with delta = 2*dp*128 and (2*dp+1)*128
    for dp in range(2):
        dj0 = (2 * dp) * P
        dj1 = (2 * dp + 1) * P
        out[:, dp, :TT] = np.where(c >= r + dj0, 0.0, -1e30)
        out[:, dp, TT:] = np.where(c >= r + dj1, 0.0, -1e30)
    return out


def _prep_inputs(x, W_attn, b_attn, W_proj):
    mask = _make_mask()
    in_maps = []
    for b in range(B):
        xaT = np.zeros((KAUG, S), dtype=np.float32)
        xaT[:C] = np.ascontiguousarray(x[b].T)
        xaT[C] = 1.0
        for hg in range(2):
            cs = hg * CG
            bqk = np.empty((P, 8), dtype=np.float32)
            for cb in range(4):
                bqk[:, cb] = b_attn[cs + cb * P : cs + (cb + 1) * P]
                bqk[:, 4 + cb] = b_attn[C + cs + cb * P : C + cs + (cb + 1) * P]
            m = {"xaT": xaT, "msk": mask, "bqk": bqk,
                 "onesd": np.ones((P, P), dtype=np.float32)}
            for name, off in (("wq", 0), ("wk", C), ("wv", 2 * C)):
                w = np.zeros((KAUG, CG), dtype=np.float32)
                w[:C] = W_attn[:, off + cs : off + cs + CG]
                w[C] = b_attn[off + cs : off + cs + CG]
                m[name] = w
            m["wp"] = np.ascontiguousarray(W_proj[cs : cs + CG, :])
            in_maps.append(m)
    return in_maps


def _run(x, W_attn, b_attn, W_proj, b_proj, trace=False, **run_kwargs):
    x = np.asarray(x, dtype=np.float32)
    W_attn = np.asarray(W_attn, dtype=np.float32)
    b_attn = np.asarray(b_attn, dtype=np.float32)
    W_proj = np.asarray(W_proj, dtype=np.float32)
    b_proj = np.asarray(b_proj, dtype=np.float32)

    nc = _get_nc()
    in_maps = _prep_inputs(x, W_attn, b_attn, W_proj)
    res = run_bass_kernel_spmd(
        nc, in_maps, core_ids=list(range(B * 2)), trace=trace, **run_kwargs
    )
    out = np.empty((B, S, C), dtype=np.float32)
    for b in range(B):
        out[b] = res.results[2 * b]["y"] + res.results[2 * b + 1]["y"] + b_proj
    return out, res


def kernel(x, W_attn, b_attn, W_proj, b_proj):
    out, _ = _run(x, W_attn, b_attn, W_proj, b_proj)
    return out
